# revision 12
# baseline (speedup 1.0000x reference)
"""Bass/Tile kernel for a single attention head, data-parallel over B=8 on
8 TRN2 NeuronCores (one batch element per core, no collectives).

Per-core problem (S=2048, D=1024, H=128):
    q = Xq @ Wq + bq ; k = Xk @ Wk + bk ; v = Xv @ Wv + bv
    out = softmax(q k^T / sqrt(H)) v

v3 layout/schedule (PE contracts over the partition dim):
  - X^T built on the HOST (numpy transpose + bf16 cast + repack) so the
    PE spends zero cycles transposing inputs.  DMA lines are 2-8KB.
  - The ACT engine's exp stream (32 ops x ~1.3us) is the serial
    bottleneck; everything is scheduled around starting it early and
    never starving it:
      * weights/biases doorbells ride the GpSimd queue (a dma_start
        costs ~680ns of issue time; serializing 18 of them on Sync was
        delaying the first k bytes by ~5us),
      * k arrives in SIXTEENTHS so scores j0 starts after only ~2.8MB
        of byte-traffic, q half 0 right before it, v last.
  - scoresT [j, i] per j-tile; exp((1/sqrt H)x) PSUM->SBUF bf16 per
    (j-tile, i-half).
  - v projected DIRECTLY to natural [s, h] (stationary X_v^T s-slice,
    moving Wv chunk; bias via a 1-partition ones-row matmul); ones
    column makes AV emit numerator + row-sums in one accumulation.
  - AV is split by i-HALF: the lower 8 i-tiles only need i-half0 exp
    (done mid-kernel) + v, so they finish and store while the exp i1
    stream still runs; only the upper half's last j-phase (~2us) trails
    the final exp.  Accumulation in one SBUF f32 tile; drains on DVE;
    normalization = batched DVE reciprocal + muls on DVE/GpSimd (lower,
    ACT still busy) and ACT/DVE/GpSimd (upper, ACT idle).
  - Output leaves as [p, itile, h] packed halves (4KB DMA lines), host
    unpacks.
"""

import sys

if "/opt/trn_rl_repo" not in sys.path:
    sys.path.insert(0, "/opt/trn_rl_repo")

import numpy as np

import concourse.bass as bass
import concourse.tile as tile
from concourse import bacc, mybir
from concourse.bass_utils import run_bass_kernel_spmd

P = 128          # partitions
S = 2048         # sequence length (per core)
D = 1024         # input dim
H = 128          # head dim (Dq = Dk)
ST = S // P      # 16 s-tiles
DC = D // P      # 8 d-chunks
NQ = 4           # s-quarters
QS = S // NQ     # 512
N_CORES = 8

F32 = mybir.dt.float32
BF16 = mybir.dt.bfloat16
AF = mybir.ActivationFunctionType

SOFTMAX_SCALE = 1.0 / float(np.sqrt(H))


def _build_kernel(tc, ins, out_ap):
    nc = tc.nc
    (qp, kp, vp, wq_ap, bq_ap, wk_ap, bk_ap, wv_ap, bv_ap) = ins

    with (
        tc.tile_pool(name="consts", bufs=1) as consts,
        tc.tile_pool(name="xq", bufs=4) as xqp,
        tc.tile_pool(name="xk", bufs=16) as xkp,
        tc.tile_pool(name="xv", bufs=4) as xvp,
        tc.tile_pool(name="proj", bufs=1) as projp,
        tc.tile_pool(name="expp", bufs=1) as expp,
        tc.tile_pool(name="vext", bufs=1) as vexp,
        tc.tile_pool(name="accp", bufs=1) as accp,
        tc.tile_pool(name="outp", bufs=1) as outp,
    ):
        # ---- tiny consts (no DMA) ----
        warm_a = consts.tile([P, P], BF16, tag="warm_a")
        nc.gpsimd.memset(warm_a, 0.5)
        ones_row = consts.tile([P, P], BF16, tag="ones_row")
        nc.gpsimd.memset(ones_row, 1.0)
        warm_sink = nc.dram_tensor("warm_sink", [P, P], F32)

        # ---- load doorbells: weights/biases on GpSimd, X on Sync, both
        # in parallel; Sync carries the byte-priority order q_h0, k, q_h1,
        # v.  (Each dma_start costs ~680ns of queue issue time.) ----
        wq = consts.tile([P, DC, H], BF16, tag="wq")
        nc.gpsimd.dma_start(out=wq, in_=wq_ap)
        bq = consts.tile([P, 1], F32, tag="bq")
        nc.gpsimd.dma_start(out=bq, in_=bq_ap)
        wk = consts.tile([P, DC, H], BF16, tag="wk")
        nc.gpsimd.dma_start(out=wk, in_=wk_ap)
        bk = consts.tile([P, 1], F32, tag="bk")
        nc.gpsimd.dma_start(out=bk, in_=bk_ap)
        wv = consts.tile([P, DC, H], BF16, tag="wv")
        nc.gpsimd.dma_start(out=wv, in_=wv_ap)
        bvr = consts.tile([1, H], BF16, tag="bvr")
        nc.gpsimd.dma_start(out=bvr, in_=bv_ap)

        xq_tiles = []
        for nq in range(2):
            xt = xqp.tile([P, DC, QS], BF16, tag="xq", name=f"xq{nq}")
            nc.sync.dma_start(out=xt, in_=qp[nq])
            xq_tiles.append(xt)
        xk_tiles = []
        for t in range(ST):
            xt = xkp.tile([P, DC, P], BF16, tag="xk", name=f"xk{t}")
            nc.sync.dma_start(out=xt, in_=kp[t])
            xk_tiles.append(xt)
        for nq in range(2, NQ):
            xt = xqp.tile([P, DC, QS], BF16, tag="xq", name=f"xq{nq}")
            nc.sync.dma_start(out=xt, in_=qp[nq])
            xq_tiles.append(xt)
        xv_tiles = []
        for nq in range(NQ):
            xt = xvp.tile([P, DC, QS], BF16, tag="xv", name=f"xv{nq}")
            nc.sync.dma_start(out=xt, in_=vp[nq])
            xv_tiles.append(xt)

        # preload the ACT exp table set (~2.7us) during DMA dead time
        dummy = consts.tile([P, 1], F32, tag="dummy")
        nc.gpsimd.memset(dummy, 0.0)
        exp_sink = consts.tile([P, 1], BF16, tag="exp_sink")
        nc.scalar.activation(exp_sink, dummy, AF.Exp, bias=0.0, scale=1.0)

        # ---- persistent SBUF tiles ----
        qTq = [
            projp.tile([P, QS], BF16, tag=f"qT{i}", name=f"qT{i}")
            for i in range(NQ)
        ]
        kT16 = [
            projp.tile([P, P], BF16, tag=f"kT{t}", name=f"kT{t}")
            for t in range(ST)
        ]
        # exp tiles split by (i-half, j-quarter) so AV phases only wait
        # on the exp they actually read
        ex = [
            [
                expp.tile([P, 4, 1024], BF16, tag=f"ex{h}{jq}", name=f"ex{h}{jq}")
                for jq in range(NQ)
            ]
            for h in range(2)
        ]
        # v natural [s,h]+ones column, split by j-quarter
        vx = [
            vexp.tile([P, 4, H + 1], BF16, tag=f"vx{jq}", name=f"vx{jq}")
            for jq in range(NQ)
        ]
        for jq in range(NQ):
            nc.gpsimd.memset(vx[jq][:, :, H : H + 1], 1.0)
        # single acc tile: all drains ride DVE in-order anyway, and one
        # tile lets the reciprocals batch 8 denominators per op
        acc = accp.tile([P, ST, H + 4], F32, tag="acc")
        rc_all = accp.tile([P, ST], F32, tag="rc_all")
        out_sb = [
            outp.tile([P, 8, H], F32, tag=f"osb{hf}", name=f"osb{hf}")
            for hf in range(2)
        ]

        with (
            tc.tile_pool(name="psS", bufs=2, space="PSUM") as psS,   # 2x2 banks
            tc.tile_pool(name="psP", bufs=2, space="PSUM") as psP,   # 2x1 banks
            tc.tile_pool(name="psB", bufs=2, space="PSUM") as psB,   # 2x1 banks
        ):
            # ---- PE warm-up: HAM clock gate needs ~3.4us of sustained
            # PE activity to release full clock; burn it pre-data ----
            ps_w = psP.tile([P, QS], F32, tag="pp", name="ps_w")
            for _ in range(28):
                nc.tensor.matmul(
                    ps_w[:, 0:P], warm_a, warm_a, start=True, stop=True
                )
            warm_sb = consts.tile([P, P], F32, tag="warm_sb")
            nc.vector.tensor_copy(warm_sb, ps_w[:, 0:P])
            nc.sync.dma_start(out=warm_sink[:, :], in_=warm_sb)

            def proj_q(nq):
                ps = psP.tile([P, QS], F32, tag="pp")
                for dc in range(DC):
                    nc.tensor.matmul(
                        ps,
                        wq[:, dc, :],
                        xq_tiles[nq][:, dc, :],
                        start=(dc == 0),
                        stop=(dc == DC - 1),
                    )
                nc.vector.tensor_scalar_add(qTq[nq], ps, bq)

            def proj_k16(t):
                ps = psP.tile([P, QS], F32, tag="pp")
                for dc in range(DC):
                    nc.tensor.matmul(
                        ps[:, 0:P],
                        wk[:, dc, :],
                        xk_tiles[t][:, dc, :],
                        start=(dc == 0),
                        stop=(dc == DC - 1),
                    )
                nc.vector.tensor_scalar_add(kT16[t], ps[:, 0:P], bk)

            def scores_exp(jt, hf):
                pss = psS.tile([P, 1024], F32, tag="ps")
                for nb in range(2):
                    nc.tensor.matmul(
                        pss[:, nb * QS : (nb + 1) * QS],
                        kT16[jt],
                        qTq[2 * hf + nb],
                        start=True,
                        stop=True,
                    )
                nc.scalar.activation(
                    ex[hf][jt // 4][:, jt % 4, :],
                    pss,
                    AF.Exp,
                    bias=0.0,
                    scale=SOFTMAX_SCALE,
                )

            def vproj_quarter(jq):
                """v natural [s,h] for s-tiles 4jq..4jq+3 into ONE psP
                tile (bias via 1-partition ones-row matmuls), drained by a
                single strided CAST — one DVE op per quarter, not four."""
                xt = xv_tiles[jq]
                ps = psP.tile([P, QS], F32, tag="pp")
                for st in range(4):
                    sl = ps[:, st * P : (st + 1) * P]
                    for dc in range(DC):
                        nc.tensor.matmul(
                            sl,
                            xt[:, dc, st * P : (st + 1) * P],
                            wv[:, dc, :],
                            start=(dc == 0),
                            stop=False,
                        )
                    nc.tensor.matmul(
                        sl, ones_row[0:1, :], bvr[0:1, :], start=False, stop=True
                    )
                nc.vector.tensor_copy(
                    vx[jq][:, :, 0:H], ps.rearrange("p (a b) -> p a b", b=P)
                )

            GROUPS = [(0, 3), (3, 3), (6, 2)]

            def av_jhalf(ihalf, ph):
                """AV partials for 8 i-tiles over j-tiles [8ph, 8ph+8),
                3 i-tiles per PSUM bank so each DVE drain moves 3 tiles."""
                i0 = 8 * ihalf
                for g0, glen in GROUPS:
                    po = psB.tile([P, 3, H + 4], F32, tag="po")
                    for m in range(glen):
                        k = g0 + m
                        for j8 in range(8):
                            jt = 8 * ph + j8
                            nc.tensor.matmul(
                                po[:, m, 0 : H + 1],
                                ex[ihalf][jt // 4][:, jt % 4, k * P : (k + 1) * P],
                                vx[jt // 4][:, jt % 4, :],
                                start=(j8 == 0),
                                stop=(j8 == 7),
                            )
                    dst = acc[:, i0 + g0 : i0 + g0 + glen, :]
                    src = po[:, 0:glen, :]
                    if ph == 0:
                        nc.vector.tensor_copy(dst, src)
                    else:
                        nc.vector.tensor_add(dst, dst, src)

            def norm_store(ihalf):
                """Batched reciprocal + 3-wide scale (broadcast rc), DVE
                only (concurrent engines on one SBUF tile thrash), then
                one packed half DMA."""
                i0 = 8 * ihalf
                nc.vector.reciprocal(
                    rc_all[:, i0 : i0 + 8],
                    acc[:, i0 : i0 + 8, H : H + 1].squeeze(-1),
                )
                for g0, glen in GROUPS:
                    rc_bc = (
                        rc_all[:, i0 + g0 : i0 + g0 + glen]
                        .unsqueeze(-1)
                        .broadcast_to([P, glen, H])
                    )
                    nc.vector.tensor_mul(
                        out_sb[ihalf][:, g0 : g0 + glen, :],
                        acc[:, i0 + g0 : i0 + g0 + glen, 0:H],
                        rc_bc,
                    )
                nc.sync.dma_start(out=out_ap[ihalf], in_=out_sb[ihalf])

            # ---- emission order == intended engine execution order ----
            proj_q(0)
            proj_q(1)
            for jt in range(ST):
                proj_k16(jt)
                scores_exp(jt, 0)
            proj_q(2)
            proj_q(3)
            for jt in range(ST):
                scores_exp(jt, 1)

            # AV: lower i-half chases v arrival (its exp is long done);
            # upper i-half chases the exp i1 stream.
            vproj_quarter(0)
            vproj_quarter(1)
            av_jhalf(0, 0)
            vproj_quarter(2)
            vproj_quarter(3)
            av_jhalf(0, 1)
            norm_store(0)            # runs while exp i1 still streams
            av_jhalf(1, 0)
            av_jhalf(1, 1)
            norm_store(1)


def build_nc():
    nc = bacc.Bacc(
        "TRN2", target_bir_lowering=False, debug=False, num_devices=N_CORES
    )
    ins = [
        nc.dram_tensor("qp", [NQ, P, DC, QS], BF16, kind="ExternalInput").ap(),
        nc.dram_tensor("kp", [ST, P, DC, P], BF16, kind="ExternalInput").ap(),
        nc.dram_tensor("vp", [NQ, P, DC, QS], BF16, kind="ExternalInput").ap(),
        nc.dram_tensor("wq", [P, DC, H], BF16, kind="ExternalInput").ap(),
        nc.dram_tensor("bq", [P, 1], F32, kind="ExternalInput").ap(),
        nc.dram_tensor("wk", [P, DC, H], BF16, kind="ExternalInput").ap(),
        nc.dram_tensor("bk", [P, 1], F32, kind="ExternalInput").ap(),
        nc.dram_tensor("wv", [P, DC, H], BF16, kind="ExternalInput").ap(),
        nc.dram_tensor("bv", [1, H], BF16, kind="ExternalInput").ap(),
    ]
    # packed [half, p, it_in_half, h]; host unpacks to [S, H]
    out_ap = nc.dram_tensor("out", [2, P, 8, H], F32, kind="ExternalOutput").ap()
    with tile.TileContext(nc) as tc:
        _build_kernel(tc, ins, out_ap)
    nc.compile()
    return nc


_NC_CACHE = None


def _get_nc():
    global _NC_CACHE
    if _NC_CACHE is None:
        _NC_CACHE = build_nc()
    return _NC_CACHE


def _pack_xt(x_f32, bf, nblk):
    """[S, D] f32 -> X^T packed [nblk, P, DC, S//nblk] bf16."""
    xt = np.ascontiguousarray(x_f32.astype(bf).T)          # [D, S]
    return np.ascontiguousarray(
        xt.reshape(DC, P, nblk, S // nblk).transpose(2, 1, 0, 3)
    )


def _pack_w(w_f32, bf):
    """[D, H] f32 -> [P, DC, H] bf16 (2KB DMA lines)."""
    return np.ascontiguousarray(
        w_f32.astype(bf).reshape(DC, P, H).transpose(1, 0, 2)
    )


def _run(inputs, trace=False, **kw):
    import ml_dtypes

    nc = _get_nc()
    bf = np.dtype(ml_dtypes.bfloat16)
    q = np.asarray(inputs["query"], dtype=np.float32)
    k = np.asarray(inputs["key"], dtype=np.float32)
    v = np.asarray(inputs["value"], dtype=np.float32)
    shared = {
        "wq": _pack_w(np.asarray(inputs["Wq"], dtype=np.float32), bf),
        "wk": _pack_w(np.asarray(inputs["Wk"], dtype=np.float32), bf),
        "wv": _pack_w(np.asarray(inputs["Wv"], dtype=np.float32), bf),
        "bq": np.ascontiguousarray(
            np.asarray(inputs["bq"], dtype=np.float32).reshape(P, 1)
        ),
        "bk": np.ascontiguousarray(
            np.asarray(inputs["bk"], dtype=np.float32).reshape(P, 1)
        ),
        "bv": np.ascontiguousarray(
            np.asarray(inputs["bv"], dtype=np.float32).astype(bf).reshape(1, H)
        ),
    }
    in_maps = [
        {
            "qp": _pack_xt(q[c], bf, NQ),
            "kp": _pack_xt(k[c], bf, ST),
            "vp": _pack_xt(v[c], bf, NQ),
            **shared,
        }
        for c in range(N_CORES)
    ]
    res = run_bass_kernel_spmd(nc, in_maps, list(range(N_CORES)), trace=trace, **kw)
    # unpack [2, P, 8, H] -> [S, H]: s = 1024*half + 128*it + p
    out = np.stack(
        [
            res.results[c]["out"].transpose(0, 2, 1, 3).reshape(S, H)
            for c in range(N_CORES)
        ],
        axis=0,
    )
    return out.astype(np.float32), res


def kernel(**inputs) -> np.ndarray:
    out, _ = _run(inputs, trace=False)
    return out


if __name__ == "__main__":
    # smoke-build only
    build_nc()
    print("build ok")


# revision 19
# speedup vs baseline: 1.0419x; 1.0419x over previous
"""Bass/Tile kernel for a single attention head, data-parallel over B=8 on
8 TRN2 NeuronCores (one batch element per core, no collectives).

Per-core problem (S=2048, D=1024, H=128):
    q = Xq @ Wq + bq ; k = Xk @ Wk + bk ; v = Xv @ Wv + bv
    out = softmax(q k^T / sqrt(H)) v

v5 design notes (PE contracts over the partition dim):
  - X^T built on the HOST (numpy transpose + bf16 cast + repack) so the
    PE spends zero cycles transposing inputs; all DMA lines are 2-8KB.
  - Every matmul pays ~LDWEIGHTS(stat cols) + N + fixed overhead, so the
    structure minimizes instruction count and maximizes N: projections
    and scores use N=512 (the PSUM-bank max for f32), k/q/v stream in
    quarters.
  - scoresT [j, i] per j-tile; exp((1/sqrt H)x) is one ACT op per
    (j-tile, i-half) PSUM->SBUF bf16.  The ACT stream (~43us) is one of
    two walls; the schedule starts it ASAP (byte-priority q half0 + k
    first) and never lets it starve (q2/q3 projections are emitted
    INSIDE the k loop; PE is in-order).
  - v projected to natural [s, h] with NO bias: since softmax rows sum
    to 1, out = num/den + bv exactly, so bv folds into the final
    normalization (scalar_tensor_tensor: (acc*rc) + bv) for free.
  - AV keeps the fused form: stationary exp^T slice [j, i-tile], moving
    v|ones [j, 129] -> numerator AND row-sums in one accumulation.
    3 i-tiles per PSUM bank; DVE drains move 3 tiles per op.  The upper
    i-half runs in j-QUARTER phases chasing the exp i1 stream so only
    ~2us of AV trails the last exp; the lower i-half (needs only early
    i0 exp + v) fills PE slack during the exp stream.
  - Output leaves as [p, itile, h] packed halves (4KB DMA lines), host
    unpacks.  Load doorbells: weights on GpSimd queue, X on Sync
    (each dma_start costs ~680ns of issue time on its queue).
"""

import sys

if "/opt/trn_rl_repo" not in sys.path:
    sys.path.insert(0, "/opt/trn_rl_repo")

import numpy as np

import concourse.bass as bass
import concourse.tile as tile
from concourse import bacc, mybir
from concourse.bass_utils import run_bass_kernel_spmd

P = 128          # partitions
S = 2048         # sequence length (per core)
D = 1024         # input dim
H = 128          # head dim (Dq = Dk)
ST = S // P      # 16 s-tiles
DC = D // P      # 8 d-chunks
NQ = 4           # s-quarters
QS = S // NQ     # 512
N_CORES = 8

F32 = mybir.dt.float32
BF16 = mybir.dt.bfloat16
AF = mybir.ActivationFunctionType

SOFTMAX_SCALE = 1.0 / float(np.sqrt(H))


def _build_kernel(tc, ins, out_ap):
    nc = tc.nc
    (qp, kp, vp, wq_ap, bq_ap, wk_ap, bk_ap, wv_ap, bv_ap) = ins

    with (
        tc.tile_pool(name="consts", bufs=1) as consts,
        tc.tile_pool(name="proj", bufs=1) as projp,
        tc.tile_pool(name="expp", bufs=1) as expp,
        tc.tile_pool(name="vext", bufs=1) as vexp,
        tc.tile_pool(name="accp", bufs=1) as accp,
        tc.tile_pool(name="outp", bufs=1) as outp,
        tc.tile_pool(name="xq", bufs=4) as xqp,
        tc.tile_pool(name="xk", bufs=4) as xkp,
        tc.tile_pool(name="xv", bufs=4) as xvp,
    ):
        # ---- tiny consts (no DMA) ----
        warm_a = consts.tile([P, P], BF16, tag="warm_a")
        nc.gpsimd.memset(warm_a, 0.5)
        warm_sink = nc.dram_tensor("warm_sink", [P, P], F32)

        # ---- load doorbells: weights/biases on GpSimd, X on Sync ----
        wq = consts.tile([P, DC, H], BF16, tag="wq")
        nc.gpsimd.dma_start(out=wq, in_=wq_ap)
        bq = consts.tile([P, 1], F32, tag="bq")
        nc.gpsimd.dma_start(out=bq, in_=bq_ap)
        wk = consts.tile([P, DC, H], BF16, tag="wk")
        nc.gpsimd.dma_start(out=wk, in_=wk_ap)
        bk = consts.tile([P, 1], F32, tag="bk")
        nc.gpsimd.dma_start(out=bk, in_=bk_ap)
        wv = consts.tile([P, DC, H], BF16, tag="wv")
        nc.gpsimd.dma_start(out=wv, in_=wv_ap)
        bvr = consts.tile([P, H], F32, tag="bvr")
        nc.gpsimd.dma_start(out=bvr, in_=bv_ap)

        xq_tiles = [
            xqp.tile([P, DC, QS], BF16, tag="xq", name=f"xq{nq}")
            for nq in range(NQ)
        ]
        xk_tiles = [
            xkp.tile([P, DC, QS], BF16, tag="xk", name=f"xk{t}")
            for t in range(NQ)
        ]
        xv_tiles = [
            xvp.tile([P, DC, QS], BF16, tag="xv", name=f"xv{nq}")
            for nq in range(NQ)
        ]
        # byte-priority: q half0 + k feed the exp stream, v is last
        nc.sync.dma_start(out=xq_tiles[0], in_=qp[0])
        nc.sync.dma_start(out=xq_tiles[1], in_=qp[1])
        for t in range(NQ):
            nc.sync.dma_start(out=xk_tiles[t], in_=kp[t])
        nc.sync.dma_start(out=xq_tiles[2], in_=qp[2])
        nc.sync.dma_start(out=xq_tiles[3], in_=qp[3])
        for t in range(NQ):
            nc.sync.dma_start(out=xv_tiles[t], in_=vp[t])

        # preload the ACT exp table set (~2.7us) during DMA dead time
        dummy = consts.tile([P, 1], F32, tag="dummy")
        nc.gpsimd.memset(dummy, 0.0)
        exp_sink = consts.tile([P, 1], BF16, tag="exp_sink")
        nc.scalar.activation(exp_sink, dummy, AF.Exp, bias=0.0, scale=1.0)

        # ---- persistent SBUF tiles ----
        qTq = [
            projp.tile([P, QS], BF16, tag=f"qT{i}", name=f"qT{i}")
            for i in range(NQ)
        ]
        kTq = [
            projp.tile([P, QS], BF16, tag=f"kT{i}", name=f"kT{i}")
            for i in range(NQ)
        ]
        ex = [
            [
                expp.tile([P, 4, 1024], BF16, tag=f"ex{h}{jq}", name=f"ex{h}{jq}")
                for jq in range(NQ)
            ]
            for h in range(2)
        ]
        vx = [
            vexp.tile([P, 4, H + 1], BF16, tag=f"vx{jq}", name=f"vx{jq}")
            for jq in range(NQ)
        ]
        for jq in range(NQ):
            nc.gpsimd.memset(vx[jq][:, :, H : H + 1], 1.0)
        acc = accp.tile([P, ST, H + 4], F32, tag="acc")
        rc_all = accp.tile([P, ST], F32, tag="rc_all")
        out_sb = [
            outp.tile([P, 8, H], F32, tag=f"osb{hf}", name=f"osb{hf}")
            for hf in range(2)
        ]

        with (
            tc.tile_pool(name="psS", bufs=2, space="PSUM") as psS,   # 2x2 banks
            tc.tile_pool(name="psP", bufs=2, space="PSUM") as psP,   # 2x1 banks
            tc.tile_pool(name="psB", bufs=2, space="PSUM") as psB,   # 2x1 banks
        ):
            # ---- PE warm-up (HAM clock ramp needs sustained activity) ----
            ps_w = psP.tile([P, QS], F32, tag="pp", name="ps_w")
            for _ in range(28):
                nc.tensor.matmul(
                    ps_w[:, 0:P], warm_a, warm_a, start=True, stop=True
                )
            warm_sb = consts.tile([P, P], F32, tag="warm_sb")
            nc.vector.tensor_copy(warm_sb, ps_w[:, 0:P])
            nc.sync.dma_start(out=warm_sink[:, :], in_=warm_sb)

            def proj_quarter(xt, w, b, dst):
                ps = psP.tile([P, QS], F32, tag="pp")
                for dc in range(DC):
                    nc.tensor.matmul(
                        ps,
                        w[:, dc, :],
                        xt[:, dc, :],
                        start=(dc == 0),
                        stop=(dc == DC - 1),
                    )
                nc.vector.tensor_scalar_add(dst, ps, b)

            def scores_exp(jt, hf):
                kt_sl = kTq[jt // 4][:, (jt % 4) * P : (jt % 4 + 1) * P]
                pss = psS.tile([P, 1024], F32, tag="ps")
                for nb in range(2):
                    nc.tensor.matmul(
                        pss[:, nb * QS : (nb + 1) * QS],
                        kt_sl,
                        qTq[2 * hf + nb],
                        start=True,
                        stop=True,
                    )
                nc.scalar.activation(
                    ex[hf][jt // 4][:, jt % 4, :],
                    pss,
                    AF.Exp,
                    bias=0.0,
                    scale=SOFTMAX_SCALE,
                )

            # v quarter projection split into s-tile emission chunks so
            # it can fill PE slack between ACT-paced scores
            vps = {}

            def vproj_stile(jq, st):
                if jq not in vps:
                    vps[jq] = psP.tile([P, QS], F32, tag="pp", name=f"vps{jq}")
                ps = vps[jq]
                for dc in range(DC):
                    nc.tensor.matmul(
                        ps[:, st * P : (st + 1) * P],
                        xv_tiles[jq][:, dc, st * P : (st + 1) * P],
                        wv[:, dc, :],
                        start=(dc == 0),
                        stop=(dc == DC - 1),
                    )

            def vdrain(jq):
                nc.vector.tensor_copy(
                    vx[jq][:, :, 0:H],
                    vps[jq].rearrange("p (a b) -> p a b", b=P),
                )

            GROUPS = [(0, 3), (3, 3), (6, 2)]

            def av_group(ihalf, j0, nj, g0, glen, first):
                """AV partials: i-tiles [8ihalf+g0, +glen) x j-tiles
                [j0, j0+nj), 3 i-tiles per PSUM bank, one DVE drain."""
                i0 = 8 * ihalf
                po = psB.tile([P, 3, H + 4], F32, tag="po")
                for m in range(glen):
                    k = g0 + m
                    for dj in range(nj):
                        jt = j0 + dj
                        nc.tensor.matmul(
                            po[:, m, 0 : H + 1],
                            ex[ihalf][jt // 4][:, jt % 4, k * P : (k + 1) * P],
                            vx[jt // 4][:, jt % 4, :],
                            start=(dj == 0),
                            stop=(dj == nj - 1),
                        )
                dst = acc[:, i0 + g0 : i0 + g0 + glen, :]
                src = po[:, 0:glen, :]
                if first:
                    nc.vector.tensor_copy(dst, src)
                else:
                    nc.vector.tensor_add(dst, dst, src)

            def norm_store(ihalf):
                """Batched reciprocal; out = acc*rc + bv (bv folds in
                free since softmax rows sum to 1); one packed half DMA."""
                i0 = 8 * ihalf
                nc.vector.reciprocal(
                    rc_all[:, i0 : i0 + 8],
                    acc[:, i0 : i0 + 8, H : H + 1].squeeze(-1),
                )
                for g0, glen in GROUPS:
                    rc_bc = (
                        rc_all[:, i0 + g0 : i0 + g0 + glen]
                        .unsqueeze(-1)
                        .broadcast_to([P, glen, H])
                    )
                    dst = out_sb[ihalf][:, g0 : g0 + glen, :]
                    nc.vector.tensor_mul(
                        dst, acc[:, i0 + g0 : i0 + g0 + glen, 0:H], rc_bc
                    )
                    bv_bc = bvr[:, :].unsqueeze(1).broadcast_to(
                        [P, glen, H]
                    )
                    nc.vector.tensor_add(dst, dst, bv_bc)
                nc.sync.dma_start(out=out_ap[ihalf], in_=out_sb[ihalf])

            # ---- emission order == intended engine execution order ----
            proj_quarter(xq_tiles[0], wq, bq, qTq[0])
            proj_quarter(xq_tiles[1], wq, bq, qTq[1])
            for kq in range(NQ):
                proj_quarter(xk_tiles[kq], wk, bk, kTq[kq])
                for jt in range(4 * kq, 4 * kq + 4):
                    scores_exp(jt, 0)
                    # fill ACT-paced slack: q half1 proj, early v s-tiles
                    if jt == 9:
                        proj_quarter(xq_tiles[2], wq, bq, qTq[2])
                    elif jt == 11:
                        proj_quarter(xq_tiles[3], wq, bq, qTq[3])
                    elif jt == 13:
                        vproj_stile(0, 0)
                        vproj_stile(0, 1)
                    elif jt == 14:
                        vproj_stile(0, 2)
                        vproj_stile(0, 3)
                        vdrain(0)
                    elif jt == 15:
                        vproj_stile(1, 0)
                        vproj_stile(1, 1)

            for jt in range(ST):
                scores_exp(jt, 1)
                if jt == 0:
                    vproj_stile(1, 2)
                    vproj_stile(1, 3)
                    vdrain(1)
                elif jt == 1:
                    vproj_stile(2, 0)
                    vproj_stile(2, 1)
                elif jt == 2:
                    vproj_stile(2, 2)
                    vproj_stile(2, 3)
                    vdrain(2)
                elif jt == 3:
                    av_group(0, 0, 8, 0, 3, True)
                elif jt == 4:
                    av_group(0, 0, 8, 3, 3, True)
                elif jt == 5:
                    av_group(0, 0, 8, 6, 2, True)
                elif jt == 6:
                    vproj_stile(3, 0)
                    vproj_stile(3, 1)
                elif jt == 7:
                    vproj_stile(3, 2)
                    vproj_stile(3, 3)
                    vdrain(3)
                elif jt == 8:
                    av_group(0, 8, 8, 0, 3, False)
                elif jt == 9:
                    av_group(0, 8, 8, 3, 3, False)
                elif jt == 10:
                    av_group(0, 8, 8, 6, 2, False)
                elif jt == 11:
                    # upper half, j-quarter 0 (exp j0-3 i1 done by now)
                    av_group(1, 0, 4, 0, 3, True)
                elif jt == 12:
                    av_group(1, 0, 4, 3, 3, True)
                    av_group(1, 0, 4, 6, 2, True)
                elif jt == 13:
                    av_group(1, 4, 4, 0, 3, False)
                    av_group(1, 4, 4, 3, 3, False)
                elif jt == 14:
                    av_group(1, 4, 4, 6, 2, False)
                elif jt == 15:
                    for g0, glen in GROUPS:
                        av_group(1, 8, 4, g0, glen, False)
            norm_store(0)
            for g0, glen in GROUPS:
                av_group(1, 12, 4, g0, glen, False)
            norm_store(1)


def build_nc():
    nc = bacc.Bacc(
        "TRN2", target_bir_lowering=False, debug=False, num_devices=N_CORES
    )
    ins = [
        nc.dram_tensor("qp", [NQ, P, DC, QS], BF16, kind="ExternalInput").ap(),
        nc.dram_tensor("kp", [NQ, P, DC, QS], BF16, kind="ExternalInput").ap(),
        nc.dram_tensor("vp", [NQ, P, DC, QS], BF16, kind="ExternalInput").ap(),
        nc.dram_tensor("wq", [P, DC, H], BF16, kind="ExternalInput").ap(),
        nc.dram_tensor("bq", [P, 1], F32, kind="ExternalInput").ap(),
        nc.dram_tensor("wk", [P, DC, H], BF16, kind="ExternalInput").ap(),
        nc.dram_tensor("bk", [P, 1], F32, kind="ExternalInput").ap(),
        nc.dram_tensor("wv", [P, DC, H], BF16, kind="ExternalInput").ap(),
        nc.dram_tensor("bv", [P, H], F32, kind="ExternalInput").ap(),
    ]
    # packed [half, p, it_in_half, h]; host unpacks to [S, H]
    out_ap = nc.dram_tensor("out", [2, P, 8, H], F32, kind="ExternalOutput").ap()
    with tile.TileContext(nc) as tc:
        _build_kernel(tc, ins, out_ap)
    nc.compile()
    return nc


_NC_CACHE = None


def _get_nc():
    global _NC_CACHE
    if _NC_CACHE is None:
        _NC_CACHE = build_nc()
    return _NC_CACHE


def _pack_xt(x_f32, bf):
    """[S, D] f32 -> X^T packed [NQ, P, DC, QS] bf16 (8KB DMA lines)."""
    xt = np.ascontiguousarray(x_f32.astype(bf).T)          # [D, S]
    return np.ascontiguousarray(
        xt.reshape(DC, P, NQ, QS).transpose(2, 1, 0, 3)
    )


def _pack_w(w_f32, bf):
    """[D, H] f32 -> [P, DC, H] bf16 (2KB DMA lines)."""
    return np.ascontiguousarray(
        w_f32.astype(bf).reshape(DC, P, H).transpose(1, 0, 2)
    )


def _run(inputs, trace=False, **kw):
    import ml_dtypes

    nc = _get_nc()
    bf = np.dtype(ml_dtypes.bfloat16)
    q = np.asarray(inputs["query"], dtype=np.float32)
    k = np.asarray(inputs["key"], dtype=np.float32)
    v = np.asarray(inputs["value"], dtype=np.float32)
    shared = {
        "wq": _pack_w(np.asarray(inputs["Wq"], dtype=np.float32), bf),
        "wk": _pack_w(np.asarray(inputs["Wk"], dtype=np.float32), bf),
        "wv": _pack_w(np.asarray(inputs["Wv"], dtype=np.float32), bf),
        "bq": np.ascontiguousarray(
            np.asarray(inputs["bq"], dtype=np.float32).reshape(P, 1)
        ),
        "bk": np.ascontiguousarray(
            np.asarray(inputs["bk"], dtype=np.float32).reshape(P, 1)
        ),
        "bv": np.ascontiguousarray(
            np.broadcast_to(
                np.asarray(inputs["bv"], dtype=np.float32).reshape(1, H), (P, H)
            )
        ),
    }
    in_maps = [
        {
            "qp": _pack_xt(q[c], bf),
            "kp": _pack_xt(k[c], bf),
            "vp": _pack_xt(v[c], bf),
            **shared,
        }
        for c in range(N_CORES)
    ]
    res = run_bass_kernel_spmd(nc, in_maps, list(range(N_CORES)), trace=trace, **kw)
    # unpack [2, P, 8, H] -> [S, H]: s = 1024*half + 128*it + p
    out = np.stack(
        [
            res.results[c]["out"].transpose(0, 2, 1, 3).reshape(S, H)
            for c in range(N_CORES)
        ],
        axis=0,
    )
    return out.astype(np.float32), res


def kernel(**inputs) -> np.ndarray:
    out, _ = _run(inputs, trace=False)
    return out


if __name__ == "__main__":
    # smoke-build only
    build_nc()
    print("build ok")


# revision 20
# speedup vs baseline: 1.1337x; 1.0881x over previous
"""Bass/Tile kernel for a single attention head, data-parallel over B=8 on
8 TRN2 NeuronCores (one batch element per core, no collectives).

Per-core problem (S=2048, D=1024, H=128):
    q = Xq @ Wq + bq ; k = Xk @ Wk + bk ; v = Xv @ Wv + bv
    out = softmax(q k^T / sqrt(H)) v

v5 design notes (PE contracts over the partition dim):
  - X^T built on the HOST (numpy transpose + bf16 cast + repack) so the
    PE spends zero cycles transposing inputs; all DMA lines are 2-8KB.
  - Every matmul pays ~LDWEIGHTS(stat cols) + N + fixed overhead, so the
    structure minimizes instruction count and maximizes N: projections
    and scores use N=512 (the PSUM-bank max for f32), k/q/v stream in
    quarters.
  - scoresT [j, i] per j-tile; exp((1/sqrt H)x) is one ACT op per
    (j-tile, i-half) PSUM->SBUF bf16.  The ACT stream (~43us) is one of
    two walls; the schedule starts it ASAP (byte-priority q half0 + k
    first) and never lets it starve (q2/q3 projections are emitted
    INSIDE the k loop; PE is in-order).
  - v projected to natural [s, h] with NO bias: since softmax rows sum
    to 1, out = num/den + bv exactly, so bv folds into the final
    normalization (scalar_tensor_tensor: (acc*rc) + bv) for free.
  - AV keeps the fused form: stationary exp^T slice [j, i-tile], moving
    v|ones [j, 129] -> numerator AND row-sums in one accumulation.
    3 i-tiles per PSUM bank; DVE drains move 3 tiles per op.  The upper
    i-half runs in j-QUARTER phases chasing the exp i1 stream so only
    ~2us of AV trails the last exp; the lower i-half (needs only early
    i0 exp + v) fills PE slack during the exp stream.
  - Output leaves as [p, itile, h] packed halves (4KB DMA lines), host
    unpacks.  Load doorbells: weights on GpSimd queue, X on Sync
    (each dma_start costs ~680ns of issue time on its queue).
"""

import sys

if "/opt/trn_rl_repo" not in sys.path:
    sys.path.insert(0, "/opt/trn_rl_repo")

import numpy as np

import concourse.bass as bass
import concourse.tile as tile
from concourse import bacc, mybir
from concourse.bass_utils import run_bass_kernel_spmd

P = 128          # partitions
S = 2048         # sequence length (per core)
D = 1024         # input dim
H = 128          # head dim (Dq = Dk)
ST = S // P      # 16 s-tiles
DC = D // P      # 8 d-chunks
NQ = 4           # s-quarters
QS = S // NQ     # 512
N_CORES = 8

F32 = mybir.dt.float32
BF16 = mybir.dt.bfloat16
AF = mybir.ActivationFunctionType

SOFTMAX_SCALE = 1.0 / float(np.sqrt(H))


def _build_kernel(tc, ins, out_ap):
    nc = tc.nc
    (qp, kp, vp, wq_ap, bq_ap, wk_ap, bk_ap, wv_ap, bv_ap) = ins

    with (
        tc.tile_pool(name="consts", bufs=1) as consts,
        tc.tile_pool(name="proj", bufs=1) as projp,
        tc.tile_pool(name="expp", bufs=1) as expp,
        tc.tile_pool(name="vext", bufs=1) as vexp,
        tc.tile_pool(name="accp", bufs=1) as accp,
        tc.tile_pool(name="outp", bufs=1) as outp,
        tc.tile_pool(name="xq", bufs=4) as xqp,
        tc.tile_pool(name="xk", bufs=4) as xkp,
        tc.tile_pool(name="xv", bufs=4) as xvp,
    ):
        # ---- tiny consts (no DMA) ----
        warm_a = consts.tile([P, P], BF16, tag="warm_a")
        nc.gpsimd.memset(warm_a, 0.5)
        warm_sink = nc.dram_tensor("warm_sink", [P, P], F32)

        # ---- load doorbells: weights/biases on GpSimd, X on Sync ----
        wq = consts.tile([P, DC, H], BF16, tag="wq")
        nc.gpsimd.dma_start(out=wq, in_=wq_ap)
        bq = consts.tile([P, 1], F32, tag="bq")
        nc.gpsimd.dma_start(out=bq, in_=bq_ap)
        wk = consts.tile([P, DC, H], BF16, tag="wk")
        nc.gpsimd.dma_start(out=wk, in_=wk_ap)
        bk = consts.tile([P, 1], F32, tag="bk")
        nc.gpsimd.dma_start(out=bk, in_=bk_ap)
        wv = consts.tile([P, DC, H], BF16, tag="wv")
        nc.gpsimd.dma_start(out=wv, in_=wv_ap)
        bvr = consts.tile([P, H], F32, tag="bvr")
        nc.gpsimd.dma_start(out=bvr, in_=bv_ap)

        xq_tiles = [
            xqp.tile([P, DC, QS], BF16, tag="xq", name=f"xq{nq}")
            for nq in range(NQ)
        ]
        xk_tiles = [
            xkp.tile([P, DC, QS], BF16, tag="xk", name=f"xk{t}")
            for t in range(NQ)
        ]
        xv_tiles = [
            xvp.tile([P, DC, QS], BF16, tag="xv", name=f"xv{nq}")
            for nq in range(NQ)
        ]
        # byte-priority: q half0 + k feed the exp stream, v is last
        nc.sync.dma_start(out=xq_tiles[0], in_=qp[0])
        nc.sync.dma_start(out=xq_tiles[1], in_=qp[1])
        for t in range(NQ):
            nc.sync.dma_start(out=xk_tiles[t], in_=kp[t])
        nc.sync.dma_start(out=xq_tiles[2], in_=qp[2])
        nc.sync.dma_start(out=xq_tiles[3], in_=qp[3])
        for t in range(NQ):
            nc.sync.dma_start(out=xv_tiles[t], in_=vp[t])

        # preload the ACT exp table set (~2.7us) during DMA dead time
        dummy = consts.tile([P, 1], F32, tag="dummy")
        nc.gpsimd.memset(dummy, 0.0)
        exp_sink = consts.tile([P, 1], BF16, tag="exp_sink")
        nc.scalar.activation(exp_sink, dummy, AF.Exp, bias=0.0, scale=1.0)

        # ---- persistent SBUF tiles ----
        qTq = [
            projp.tile([P, QS], BF16, tag=f"qT{i}", name=f"qT{i}")
            for i in range(NQ)
        ]
        kTq = [
            projp.tile([P, QS], BF16, tag=f"kT{i}", name=f"kT{i}")
            for i in range(NQ)
        ]
        ex = [
            [
                expp.tile([P, 4, 1024], BF16, tag=f"ex{h}{jq}", name=f"ex{h}{jq}")
                for jq in range(NQ)
            ]
            for h in range(2)
        ]
        vx = [
            vexp.tile([P, 4, H + 1], BF16, tag=f"vx{jq}", name=f"vx{jq}")
            for jq in range(NQ)
        ]
        for jq in range(NQ):
            nc.gpsimd.memset(vx[jq][:, :, H : H + 1], 1.0)
        acc = accp.tile([P, ST, H + 4], F32, tag="acc")
        rc_all = accp.tile([P, ST], F32, tag="rc_all")
        out_sb = [
            outp.tile([P, 8, H], F32, tag=f"osb{hf}", name=f"osb{hf}")
            for hf in range(2)
        ]

        with (
            tc.tile_pool(name="psS", bufs=2, space="PSUM") as psS,   # 2x2 banks
            tc.tile_pool(name="psP", bufs=2, space="PSUM") as psP,   # 2x1 banks
            tc.tile_pool(name="psB", bufs=2, space="PSUM") as psB,   # 2x1 banks
        ):
            # ---- PE warm-up (HAM clock ramp needs sustained activity) ----
            ps_w = psP.tile([P, QS], F32, tag="pp", name="ps_w")
            for _ in range(14):
                nc.tensor.matmul(
                    ps_w[:, 0:P], warm_a, warm_a, start=True, stop=True
                )
            warm_sb = consts.tile([P, P], F32, tag="warm_sb")
            nc.vector.tensor_copy(warm_sb, ps_w[:, 0:P])
            nc.sync.dma_start(out=warm_sink[:, :], in_=warm_sb)

            def proj_quarter(xt, w, b, dst):
                ps = psP.tile([P, QS], F32, tag="pp")
                for dc in range(DC):
                    nc.tensor.matmul(
                        ps,
                        w[:, dc, :],
                        xt[:, dc, :],
                        start=(dc == 0),
                        stop=(dc == DC - 1),
                    )
                nc.vector.tensor_scalar_add(dst, ps, b)

            def scores_exp(jt, hf):
                kt_sl = kTq[jt // 4][:, (jt % 4) * P : (jt % 4 + 1) * P]
                pss = psS.tile([P, 1024], F32, tag="ps")
                for nb in range(2):
                    nc.tensor.matmul(
                        pss[:, nb * QS : (nb + 1) * QS],
                        kt_sl,
                        qTq[2 * hf + nb],
                        start=True,
                        stop=True,
                    )
                nc.scalar.activation(
                    ex[hf][jt // 4][:, jt % 4, :],
                    pss,
                    AF.Exp,
                    bias=0.0,
                    scale=SOFTMAX_SCALE,
                )

            # v quarter projection split into s-tile emission chunks so
            # it can fill PE slack between ACT-paced scores
            vps = {}

            def vproj_stile(jq, st):
                if jq not in vps:
                    vps[jq] = psP.tile([P, QS], F32, tag="pp", name=f"vps{jq}")
                ps = vps[jq]
                for dc in range(DC):
                    nc.tensor.matmul(
                        ps[:, st * P : (st + 1) * P],
                        xv_tiles[jq][:, dc, st * P : (st + 1) * P],
                        wv[:, dc, :],
                        start=(dc == 0),
                        stop=(dc == DC - 1),
                    )

            def vdrain(jq):
                nc.vector.tensor_copy(
                    vx[jq][:, :, 0:H],
                    vps[jq].rearrange("p (a b) -> p a b", b=P),
                )

            GROUPS = [(0, 3), (3, 3), (6, 2)]

            def av_group(ihalf, j0, nj, g0, glen, first):
                """AV partials: i-tiles [8ihalf+g0, +glen) x j-tiles
                [j0, j0+nj), 3 i-tiles per PSUM bank, one DVE drain."""
                i0 = 8 * ihalf
                po = psB.tile([P, 3, H + 4], F32, tag="po")
                for m in range(glen):
                    k = g0 + m
                    for dj in range(nj):
                        jt = j0 + dj
                        nc.tensor.matmul(
                            po[:, m, 0 : H + 1],
                            ex[ihalf][jt // 4][:, jt % 4, k * P : (k + 1) * P],
                            vx[jt // 4][:, jt % 4, :],
                            start=(dj == 0),
                            stop=(dj == nj - 1),
                        )
                dst = acc[:, i0 + g0 : i0 + g0 + glen, :]
                src = po[:, 0:glen, :]
                if first:
                    nc.vector.tensor_copy(dst, src)
                else:
                    nc.vector.tensor_add(dst, dst, src)

            def norm_store(ihalf):
                """Batched reciprocal; out = acc*rc + bv (bv folds in
                free since softmax rows sum to 1); one packed half DMA."""
                i0 = 8 * ihalf
                nc.vector.reciprocal(
                    rc_all[:, i0 : i0 + 8],
                    acc[:, i0 : i0 + 8, H : H + 1].squeeze(-1),
                )
                for g0, glen in GROUPS:
                    rc_bc = (
                        rc_all[:, i0 + g0 : i0 + g0 + glen]
                        .unsqueeze(-1)
                        .broadcast_to([P, glen, H])
                    )
                    dst = out_sb[ihalf][:, g0 : g0 + glen, :]
                    nc.vector.tensor_mul(
                        dst, acc[:, i0 + g0 : i0 + g0 + glen, 0:H], rc_bc
                    )
                    bv_bc = bvr[:, :].unsqueeze(1).broadcast_to(
                        [P, glen, H]
                    )
                    nc.vector.tensor_add(dst, dst, bv_bc)
                nc.sync.dma_start(out=out_ap[ihalf], in_=out_sb[ihalf])

            # ---- emission order == intended engine execution order ----
            # PE is in-order: every insertion is placed at the point
            # where its data has just arrived, sized ~<=2us so the
            # ACT-paced scores stream never starves for long.
            proj_quarter(xq_tiles[0], wq, bq, qTq[0])
            proj_quarter(xq_tiles[1], wq, bq, qTq[1])
            for kq in range(NQ):
                proj_quarter(xk_tiles[kq], wk, bk, kTq[kq])
                for jt in range(4 * kq, 4 * kq + 4):
                    scores_exp(jt, 0)
                    if jt == 11:
                        proj_quarter(xq_tiles[2], wq, bq, qTq[2])
                    elif jt == 12:
                        proj_quarter(xq_tiles[3], wq, bq, qTq[3])
                    elif jt == 13:
                        vproj_stile(0, 0)
                        vproj_stile(0, 1)
                    elif jt == 14:
                        vproj_stile(0, 2)
                        vproj_stile(0, 3)
                        vdrain(0)
                    elif jt == 15:
                        vproj_stile(1, 0)
                        vproj_stile(1, 1)

            for jt in range(ST):
                scores_exp(jt, 1)
                if jt == 0:
                    vproj_stile(1, 2)
                    vproj_stile(1, 3)
                    vdrain(1)
                elif jt == 1:
                    vproj_stile(2, 0)
                    vproj_stile(2, 1)
                elif jt == 2:
                    vproj_stile(2, 2)
                    vproj_stile(2, 3)
                    vdrain(2)
                elif jt == 3:
                    av_group(0, 0, 8, 0, 3, True)
                elif jt == 4:
                    av_group(0, 0, 8, 3, 3, True)
                elif jt == 5:
                    av_group(0, 0, 8, 6, 2, True)
                elif jt == 6:
                    vproj_stile(3, 0)
                    vproj_stile(3, 1)
                elif jt == 7:
                    vproj_stile(3, 2)
                    vproj_stile(3, 3)
                    vdrain(3)
                elif jt == 8:
                    av_group(0, 8, 8, 0, 3, False)
                elif jt == 9:
                    av_group(0, 8, 8, 3, 3, False)
                elif jt == 10:
                    av_group(0, 8, 8, 6, 2, False)
                elif jt == 11:
                    av_group(1, 0, 4, 0, 3, True)
                    av_group(1, 0, 4, 3, 3, True)
                elif jt == 12:
                    av_group(1, 0, 4, 6, 2, True)
                    av_group(1, 4, 4, 0, 3, False)
                elif jt == 13:
                    av_group(1, 4, 4, 3, 3, False)
                    av_group(1, 4, 4, 6, 2, False)
                elif jt == 14:
                    av_group(1, 8, 4, 0, 3, False)
                    av_group(1, 8, 4, 3, 3, False)
                elif jt == 15:
                    av_group(1, 8, 4, 6, 2, False)
            norm_store(0)
            for g0, glen in GROUPS:
                av_group(1, 12, 4, g0, glen, False)
            norm_store(1)


def build_nc():
    nc = bacc.Bacc(
        "TRN2", target_bir_lowering=False, debug=False, num_devices=N_CORES
    )
    ins = [
        nc.dram_tensor("qp", [NQ, P, DC, QS], BF16, kind="ExternalInput").ap(),
        nc.dram_tensor("kp", [NQ, P, DC, QS], BF16, kind="ExternalInput").ap(),
        nc.dram_tensor("vp", [NQ, P, DC, QS], BF16, kind="ExternalInput").ap(),
        nc.dram_tensor("wq", [P, DC, H], BF16, kind="ExternalInput").ap(),
        nc.dram_tensor("bq", [P, 1], F32, kind="ExternalInput").ap(),
        nc.dram_tensor("wk", [P, DC, H], BF16, kind="ExternalInput").ap(),
        nc.dram_tensor("bk", [P, 1], F32, kind="ExternalInput").ap(),
        nc.dram_tensor("wv", [P, DC, H], BF16, kind="ExternalInput").ap(),
        nc.dram_tensor("bv", [P, H], F32, kind="ExternalInput").ap(),
    ]
    # packed [half, p, it_in_half, h]; host unpacks to [S, H]
    out_ap = nc.dram_tensor("out", [2, P, 8, H], F32, kind="ExternalOutput").ap()
    with tile.TileContext(nc) as tc:
        _build_kernel(tc, ins, out_ap)
    nc.compile()
    return nc


_NC_CACHE = None


def _get_nc():
    global _NC_CACHE
    if _NC_CACHE is None:
        _NC_CACHE = build_nc()
    return _NC_CACHE


def _pack_xt(x_f32, bf):
    """[S, D] f32 -> X^T packed [NQ, P, DC, QS] bf16 (8KB DMA lines)."""
    xt = np.ascontiguousarray(x_f32.astype(bf).T)          # [D, S]
    return np.ascontiguousarray(
        xt.reshape(DC, P, NQ, QS).transpose(2, 1, 0, 3)
    )


def _pack_w(w_f32, bf):
    """[D, H] f32 -> [P, DC, H] bf16 (2KB DMA lines)."""
    return np.ascontiguousarray(
        w_f32.astype(bf).reshape(DC, P, H).transpose(1, 0, 2)
    )


def _run(inputs, trace=False, **kw):
    import ml_dtypes

    nc = _get_nc()
    bf = np.dtype(ml_dtypes.bfloat16)
    q = np.asarray(inputs["query"], dtype=np.float32)
    k = np.asarray(inputs["key"], dtype=np.float32)
    v = np.asarray(inputs["value"], dtype=np.float32)
    shared = {
        "wq": _pack_w(np.asarray(inputs["Wq"], dtype=np.float32), bf),
        "wk": _pack_w(np.asarray(inputs["Wk"], dtype=np.float32), bf),
        "wv": _pack_w(np.asarray(inputs["Wv"], dtype=np.float32), bf),
        "bq": np.ascontiguousarray(
            np.asarray(inputs["bq"], dtype=np.float32).reshape(P, 1)
        ),
        "bk": np.ascontiguousarray(
            np.asarray(inputs["bk"], dtype=np.float32).reshape(P, 1)
        ),
        "bv": np.ascontiguousarray(
            np.broadcast_to(
                np.asarray(inputs["bv"], dtype=np.float32).reshape(1, H), (P, H)
            )
        ),
    }
    in_maps = [
        {
            "qp": _pack_xt(q[c], bf),
            "kp": _pack_xt(k[c], bf),
            "vp": _pack_xt(v[c], bf),
            **shared,
        }
        for c in range(N_CORES)
    ]
    res = run_bass_kernel_spmd(nc, in_maps, list(range(N_CORES)), trace=trace, **kw)
    # unpack [2, P, 8, H] -> [S, H]: s = 1024*half + 128*it + p
    out = np.stack(
        [
            res.results[c]["out"].transpose(0, 2, 1, 3).reshape(S, H)
            for c in range(N_CORES)
        ],
        axis=0,
    )
    return out.astype(np.float32), res


def kernel(**inputs) -> np.ndarray:
    out, _ = _run(inputs, trace=False)
    return out


if __name__ == "__main__":
    # smoke-build only
    build_nc()
    print("build ok")


# revision 21
# speedup vs baseline: 1.1563x; 1.0199x over previous
"""Bass/Tile kernel for a single attention head, data-parallel over B=8 on
8 TRN2 NeuronCores (one batch element per core, no collectives).

Per-core problem (S=2048, D=1024, H=128):
    q = Xq @ Wq + bq ; k = Xk @ Wk + bk ; v = Xv @ Wv + bv
    out = softmax(q k^T / sqrt(H)) v

v5 design notes (PE contracts over the partition dim):
  - X^T built on the HOST (numpy transpose + bf16 cast + repack) so the
    PE spends zero cycles transposing inputs; all DMA lines are 2-8KB.
  - Every matmul pays ~LDWEIGHTS(stat cols) + N + fixed overhead, so the
    structure minimizes instruction count and maximizes N: projections
    and scores use N=512 (the PSUM-bank max for f32), k/q/v stream in
    quarters.
  - scoresT [j, i] per j-tile; exp((1/sqrt H)x) is one ACT op per
    (j-tile, i-half) PSUM->SBUF bf16.  The ACT stream (~43us) is one of
    two walls; the schedule starts it ASAP (byte-priority q half0 + k
    first) and never lets it starve (q2/q3 projections are emitted
    INSIDE the k loop; PE is in-order).
  - v projected to natural [s, h] with NO bias: since softmax rows sum
    to 1, out = num/den + bv exactly, so bv folds into the final
    normalization (scalar_tensor_tensor: (acc*rc) + bv) for free.
  - AV keeps the fused form: stationary exp^T slice [j, i-tile], moving
    v|ones [j, 129] -> numerator AND row-sums in one accumulation.
    3 i-tiles per PSUM bank; DVE drains move 3 tiles per op.  The upper
    i-half runs in j-QUARTER phases chasing the exp i1 stream so only
    ~2us of AV trails the last exp; the lower i-half (needs only early
    i0 exp + v) fills PE slack during the exp stream.
  - Output leaves as [p, itile, h] packed halves (4KB DMA lines), host
    unpacks.  Load doorbells: weights on GpSimd queue, X on Sync
    (each dma_start costs ~680ns of issue time on its queue).
"""

import sys

if "/opt/trn_rl_repo" not in sys.path:
    sys.path.insert(0, "/opt/trn_rl_repo")

import numpy as np

import concourse.bass as bass
import concourse.tile as tile
from concourse import bacc, mybir
from concourse.bass_utils import run_bass_kernel_spmd

P = 128          # partitions
S = 2048         # sequence length (per core)
D = 1024         # input dim
H = 128          # head dim (Dq = Dk)
ST = S // P      # 16 s-tiles
DC = D // P      # 8 d-chunks
NQ = 4           # s-quarters
QS = S // NQ     # 512
N_CORES = 8

F32 = mybir.dt.float32
BF16 = mybir.dt.bfloat16
AF = mybir.ActivationFunctionType

SOFTMAX_SCALE = 1.0 / float(np.sqrt(H))


def _build_kernel(tc, ins, out_ap):
    nc = tc.nc
    (qp, kp, vp, wq_ap, bq_ap, wk_ap, bk_ap, wv_ap, bv_ap) = ins

    with (
        tc.tile_pool(name="consts", bufs=1) as consts,
        tc.tile_pool(name="proj", bufs=1) as projp,
        tc.tile_pool(name="expp", bufs=1) as expp,
        tc.tile_pool(name="vext", bufs=1) as vexp,
        tc.tile_pool(name="accp", bufs=1) as accp,
        tc.tile_pool(name="outp", bufs=1) as outp,
        tc.tile_pool(name="xq", bufs=4) as xqp,
        tc.tile_pool(name="xk", bufs=4) as xkp,
        tc.tile_pool(name="xv", bufs=4) as xvp,
    ):
        # ---- tiny consts (no DMA) ----
        warm_a = consts.tile([P, P], BF16, tag="warm_a")
        nc.gpsimd.memset(warm_a, 0.5)
        warm_sink = nc.dram_tensor("warm_sink", [P, P], F32)

        # ---- load doorbells: weights/biases on GpSimd, X on Sync ----
        wq = consts.tile([P, DC, H], BF16, tag="wq")
        nc.gpsimd.dma_start(out=wq, in_=wq_ap)
        bq = consts.tile([P, 1], F32, tag="bq")
        nc.gpsimd.dma_start(out=bq, in_=bq_ap)
        wk = consts.tile([P, DC, H], BF16, tag="wk")
        nc.gpsimd.dma_start(out=wk, in_=wk_ap)
        bk = consts.tile([P, 1], F32, tag="bk")
        nc.gpsimd.dma_start(out=bk, in_=bk_ap)
        wv = consts.tile([P, DC, H], BF16, tag="wv")
        nc.gpsimd.dma_start(out=wv, in_=wv_ap)
        bvr = consts.tile([P, H], F32, tag="bvr")
        nc.gpsimd.dma_start(out=bvr, in_=bv_ap)

        xq_tiles = [
            xqp.tile([P, DC, QS], BF16, tag="xq", name=f"xq{nq}")
            for nq in range(NQ)
        ]
        xk_tiles = [
            xkp.tile([P, DC, QS], BF16, tag="xk", name=f"xk{t}")
            for t in range(NQ)
        ]
        xv_tiles = [
            xvp.tile([P, DC, QS], BF16, tag="xv", name=f"xv{nq}")
            for nq in range(NQ)
        ]
        # byte-priority: q half0 + k feed the exp stream, v is last
        nc.sync.dma_start(out=xq_tiles[0], in_=qp[0])
        nc.sync.dma_start(out=xq_tiles[1], in_=qp[1])
        for t in range(NQ):
            nc.sync.dma_start(out=xk_tiles[t], in_=kp[t])
        nc.sync.dma_start(out=xq_tiles[2], in_=qp[2])
        nc.sync.dma_start(out=xq_tiles[3], in_=qp[3])
        for t in range(NQ):
            nc.sync.dma_start(out=xv_tiles[t], in_=vp[t])

        # preload the ACT exp table set (~2.7us) during DMA dead time
        dummy = consts.tile([P, 1], F32, tag="dummy")
        nc.gpsimd.memset(dummy, 0.0)
        exp_sink = consts.tile([P, 1], BF16, tag="exp_sink")
        nc.scalar.activation(exp_sink, dummy, AF.Exp, bias=0.0, scale=1.0)

        # ---- persistent SBUF tiles ----
        qTq = [
            projp.tile([P, QS], BF16, tag=f"qT{i}", name=f"qT{i}")
            for i in range(NQ)
        ]
        kTq = [
            projp.tile([P, QS], BF16, tag=f"kT{i}", name=f"kT{i}")
            for i in range(NQ)
        ]
        ex = [
            [
                expp.tile([P, 4, 1024], BF16, tag=f"ex{h}{jq}", name=f"ex{h}{jq}")
                for jq in range(NQ)
            ]
            for h in range(2)
        ]
        vx = [
            vexp.tile([P, 4, H + 1], BF16, tag=f"vx{jq}", name=f"vx{jq}")
            for jq in range(NQ)
        ]
        for jq in range(NQ):
            nc.gpsimd.memset(vx[jq][:, :, H : H + 1], 1.0)
        acc = accp.tile([P, ST, H + 4], F32, tag="acc")
        rc_all = accp.tile([P, ST], F32, tag="rc_all")
        out_sb = [
            outp.tile([P, 8, H], F32, tag=f"osb{hf}", name=f"osb{hf}")
            for hf in range(2)
        ]

        with (
            tc.tile_pool(name="psS", bufs=2, space="PSUM") as psS,   # 2x2 banks
            tc.tile_pool(name="psP", bufs=2, space="PSUM") as psP,   # 2x1 banks
            tc.tile_pool(name="psB", bufs=2, space="PSUM") as psB,   # 2x1 banks
        ):
            # ---- PE warm-up (HAM clock ramp needs sustained activity) ----
            ps_w = psP.tile([P, QS], F32, tag="pp", name="ps_w")
            for _ in range(90):
                nc.tensor.matmul(
                    ps_w[:, 0:P], warm_a, warm_a, start=True, stop=True
                )
            warm_sb = consts.tile([P, P], F32, tag="warm_sb")
            nc.vector.tensor_copy(warm_sb, ps_w[:, 0:P])
            nc.sync.dma_start(out=warm_sink[:, :], in_=warm_sb)

            def pe_keepalive(n):
                for _ in range(n):
                    nc.tensor.matmul(
                        ps_w[:, 0:P], warm_a, warm_a, start=True, stop=True
                    )

            def proj_quarter(xt, w, b, dst):
                ps = psP.tile([P, QS], F32, tag="pp")
                for dc in range(DC):
                    nc.tensor.matmul(
                        ps,
                        w[:, dc, :],
                        xt[:, dc, :],
                        start=(dc == 0),
                        stop=(dc == DC - 1),
                    )
                nc.vector.tensor_scalar_add(dst, ps, b)

            def scores_exp(jt, hf):
                kt_sl = kTq[jt // 4][:, (jt % 4) * P : (jt % 4 + 1) * P]
                pss = psS.tile([P, 1024], F32, tag="ps")
                for nb in range(2):
                    nc.tensor.matmul(
                        pss[:, nb * QS : (nb + 1) * QS],
                        kt_sl,
                        qTq[2 * hf + nb],
                        start=True,
                        stop=True,
                    )
                nc.scalar.activation(
                    ex[hf][jt // 4][:, jt % 4, :],
                    pss,
                    AF.Exp,
                    bias=0.0,
                    scale=SOFTMAX_SCALE,
                )

            # v quarter projection split into s-tile emission chunks so
            # it can fill PE slack between ACT-paced scores
            vps = {}

            def vproj_stile(jq, st):
                if jq not in vps:
                    vps[jq] = psP.tile([P, QS], F32, tag="pp", name=f"vps{jq}")
                ps = vps[jq]
                for dc in range(DC):
                    nc.tensor.matmul(
                        ps[:, st * P : (st + 1) * P],
                        xv_tiles[jq][:, dc, st * P : (st + 1) * P],
                        wv[:, dc, :],
                        start=(dc == 0),
                        stop=(dc == DC - 1),
                    )

            def vdrain(jq):
                nc.vector.tensor_copy(
                    vx[jq][:, :, 0:H],
                    vps[jq].rearrange("p (a b) -> p a b", b=P),
                )

            GROUPS = [(0, 3), (3, 3), (6, 2)]

            def av_group(ihalf, j0, nj, g0, glen, first):
                """AV partials: i-tiles [8ihalf+g0, +glen) x j-tiles
                [j0, j0+nj), 3 i-tiles per PSUM bank, one DVE drain."""
                i0 = 8 * ihalf
                po = psB.tile([P, 3, H + 4], F32, tag="po")
                for m in range(glen):
                    k = g0 + m
                    for dj in range(nj):
                        jt = j0 + dj
                        nc.tensor.matmul(
                            po[:, m, 0 : H + 1],
                            ex[ihalf][jt // 4][:, jt % 4, k * P : (k + 1) * P],
                            vx[jt // 4][:, jt % 4, :],
                            start=(dj == 0),
                            stop=(dj == nj - 1),
                        )
                dst = acc[:, i0 + g0 : i0 + g0 + glen, :]
                src = po[:, 0:glen, :]
                if first:
                    nc.vector.tensor_copy(dst, src)
                else:
                    nc.vector.tensor_add(dst, dst, src)

            def norm_store(ihalf):
                """Batched reciprocal; out = acc*rc + bv (bv folds in
                free since softmax rows sum to 1); one packed half DMA."""
                i0 = 8 * ihalf
                nc.vector.reciprocal(
                    rc_all[:, i0 : i0 + 8],
                    acc[:, i0 : i0 + 8, H : H + 1].squeeze(-1),
                )
                for g0, glen in GROUPS:
                    rc_bc = (
                        rc_all[:, i0 + g0 : i0 + g0 + glen]
                        .unsqueeze(-1)
                        .broadcast_to([P, glen, H])
                    )
                    dst = out_sb[ihalf][:, g0 : g0 + glen, :]
                    nc.vector.tensor_mul(
                        dst, acc[:, i0 + g0 : i0 + g0 + glen, 0:H], rc_bc
                    )
                    bv_bc = bvr[:, :].unsqueeze(1).broadcast_to(
                        [P, glen, H]
                    )
                    nc.vector.tensor_add(dst, dst, bv_bc)
                nc.sync.dma_start(out=out_ap[ihalf], in_=out_sb[ihalf])

            # ---- emission order == intended engine execution order ----
            # PE is in-order: every insertion is placed at the point
            # where its data has just arrived, sized ~<=2us so the
            # ACT-paced scores stream never starves for long.
            proj_quarter(xq_tiles[0], wq, bq, qTq[0])
            proj_quarter(xq_tiles[1], wq, bq, qTq[1])
            pe_keepalive(20)
            for kq in range(NQ):
                proj_quarter(xk_tiles[kq], wk, bk, kTq[kq])
                for jt in range(4 * kq, 4 * kq + 4):
                    scores_exp(jt, 0)
                    if jt == 10:
                        proj_quarter(xq_tiles[2], wq, bq, qTq[2])
                    elif jt == 12:
                        proj_quarter(xq_tiles[3], wq, bq, qTq[3])
                    elif jt == 13:
                        vproj_stile(0, 0)
                        vproj_stile(0, 1)
                    elif jt == 14:
                        vproj_stile(0, 2)
                        vproj_stile(0, 3)
                        vdrain(0)
                    elif jt == 15:
                        vproj_stile(1, 0)
                        vproj_stile(1, 1)

            for jt in range(ST):
                scores_exp(jt, 1)
                if jt == 0:
                    vproj_stile(1, 2)
                    vproj_stile(1, 3)
                    vdrain(1)
                elif jt == 1:
                    vproj_stile(2, 0)
                    vproj_stile(2, 1)
                elif jt == 2:
                    vproj_stile(2, 2)
                    vproj_stile(2, 3)
                    vdrain(2)
                elif jt == 3:
                    av_group(0, 0, 8, 0, 3, True)
                elif jt == 4:
                    av_group(0, 0, 8, 3, 3, True)
                elif jt == 5:
                    av_group(0, 0, 8, 6, 2, True)
                elif jt == 6:
                    vproj_stile(3, 0)
                    vproj_stile(3, 1)
                elif jt == 7:
                    vproj_stile(3, 2)
                    vproj_stile(3, 3)
                    vdrain(3)
                elif jt == 8:
                    av_group(0, 8, 8, 0, 3, False)
                elif jt == 9:
                    av_group(0, 8, 8, 3, 3, False)
                elif jt == 10:
                    av_group(0, 8, 8, 6, 2, False)
                elif jt == 11:
                    av_group(1, 0, 4, 0, 3, True)
                elif jt == 12:
                    av_group(1, 0, 4, 3, 3, True)
                elif jt == 13:
                    av_group(1, 0, 4, 6, 2, True)
                    av_group(1, 4, 4, 0, 3, False)
                elif jt == 14:
                    av_group(1, 4, 4, 3, 3, False)
                elif jt == 15:
                    av_group(1, 4, 4, 6, 2, False)
            for g0, glen in GROUPS:
                av_group(1, 8, 4, g0, glen, False)
            norm_store(0)
            for g0, glen in GROUPS:
                av_group(1, 12, 4, g0, glen, False)
            norm_store(1)


def build_nc():
    nc = bacc.Bacc(
        "TRN2", target_bir_lowering=False, debug=False, num_devices=N_CORES
    )
    ins = [
        nc.dram_tensor("qp", [NQ, P, DC, QS], BF16, kind="ExternalInput").ap(),
        nc.dram_tensor("kp", [NQ, P, DC, QS], BF16, kind="ExternalInput").ap(),
        nc.dram_tensor("vp", [NQ, P, DC, QS], BF16, kind="ExternalInput").ap(),
        nc.dram_tensor("wq", [P, DC, H], BF16, kind="ExternalInput").ap(),
        nc.dram_tensor("bq", [P, 1], F32, kind="ExternalInput").ap(),
        nc.dram_tensor("wk", [P, DC, H], BF16, kind="ExternalInput").ap(),
        nc.dram_tensor("bk", [P, 1], F32, kind="ExternalInput").ap(),
        nc.dram_tensor("wv", [P, DC, H], BF16, kind="ExternalInput").ap(),
        nc.dram_tensor("bv", [P, H], F32, kind="ExternalInput").ap(),
    ]
    # packed [half, p, it_in_half, h]; host unpacks to [S, H]
    out_ap = nc.dram_tensor("out", [2, P, 8, H], F32, kind="ExternalOutput").ap()
    with tile.TileContext(nc) as tc:
        _build_kernel(tc, ins, out_ap)
    nc.compile()
    return nc


_NC_CACHE = None


def _get_nc():
    global _NC_CACHE
    if _NC_CACHE is None:
        _NC_CACHE = build_nc()
    return _NC_CACHE


def _pack_xt(x_f32, bf):
    """[S, D] f32 -> X^T packed [NQ, P, DC, QS] bf16 (8KB DMA lines)."""
    xt = np.ascontiguousarray(x_f32.astype(bf).T)          # [D, S]
    return np.ascontiguousarray(
        xt.reshape(DC, P, NQ, QS).transpose(2, 1, 0, 3)
    )


def _pack_w(w_f32, bf):
    """[D, H] f32 -> [P, DC, H] bf16 (2KB DMA lines)."""
    return np.ascontiguousarray(
        w_f32.astype(bf).reshape(DC, P, H).transpose(1, 0, 2)
    )


def _run(inputs, trace=False, **kw):
    import ml_dtypes

    nc = _get_nc()
    bf = np.dtype(ml_dtypes.bfloat16)
    q = np.asarray(inputs["query"], dtype=np.float32)
    k = np.asarray(inputs["key"], dtype=np.float32)
    v = np.asarray(inputs["value"], dtype=np.float32)
    shared = {
        "wq": _pack_w(np.asarray(inputs["Wq"], dtype=np.float32), bf),
        "wk": _pack_w(np.asarray(inputs["Wk"], dtype=np.float32), bf),
        "wv": _pack_w(np.asarray(inputs["Wv"], dtype=np.float32), bf),
        "bq": np.ascontiguousarray(
            np.asarray(inputs["bq"], dtype=np.float32).reshape(P, 1)
        ),
        "bk": np.ascontiguousarray(
            np.asarray(inputs["bk"], dtype=np.float32).reshape(P, 1)
        ),
        "bv": np.ascontiguousarray(
            np.broadcast_to(
                np.asarray(inputs["bv"], dtype=np.float32).reshape(1, H), (P, H)
            )
        ),
    }
    in_maps = [
        {
            "qp": _pack_xt(q[c], bf),
            "kp": _pack_xt(k[c], bf),
            "vp": _pack_xt(v[c], bf),
            **shared,
        }
        for c in range(N_CORES)
    ]
    res = run_bass_kernel_spmd(nc, in_maps, list(range(N_CORES)), trace=trace, **kw)
    # unpack [2, P, 8, H] -> [S, H]: s = 1024*half + 128*it + p
    out = np.stack(
        [
            res.results[c]["out"].transpose(0, 2, 1, 3).reshape(S, H)
            for c in range(N_CORES)
        ],
        axis=0,
    )
    return out.astype(np.float32), res


def kernel(**inputs) -> np.ndarray:
    out, _ = _run(inputs, trace=False)
    return out


if __name__ == "__main__":
    # smoke-build only
    build_nc()
    print("build ok")


# revision 22
# speedup vs baseline: 1.1843x; 1.0242x over previous
"""Bass/Tile kernel for a single attention head, data-parallel over B=8 on
8 TRN2 NeuronCores (one batch element per core, no collectives).

Per-core problem (S=2048, D=1024, H=128):
    q = Xq @ Wq + bq ; k = Xk @ Wk + bk ; v = Xv @ Wv + bv
    out = softmax(q k^T / sqrt(H)) v

v5 design notes (PE contracts over the partition dim):
  - X^T built on the HOST (numpy transpose + bf16 cast + repack) so the
    PE spends zero cycles transposing inputs; all DMA lines are 2-8KB.
  - Every matmul pays ~LDWEIGHTS(stat cols) + N + fixed overhead, so the
    structure minimizes instruction count and maximizes N: projections
    and scores use N=512 (the PSUM-bank max for f32), k/q/v stream in
    quarters.
  - scoresT [j, i] per j-tile; exp((1/sqrt H)x) is one ACT op per
    (j-tile, i-half) PSUM->SBUF bf16.  The ACT stream (~43us) is one of
    two walls; the schedule starts it ASAP (byte-priority q half0 + k
    first) and never lets it starve (q2/q3 projections are emitted
    INSIDE the k loop; PE is in-order).
  - v projected to natural [s, h] with NO bias: since softmax rows sum
    to 1, out = num/den + bv exactly, so bv folds into the final
    normalization (scalar_tensor_tensor: (acc*rc) + bv) for free.
  - AV keeps the fused form: stationary exp^T slice [j, i-tile], moving
    v|ones [j, 129] -> numerator AND row-sums in one accumulation.
    3 i-tiles per PSUM bank; DVE drains move 3 tiles per op.  The upper
    i-half runs in j-QUARTER phases chasing the exp i1 stream so only
    ~2us of AV trails the last exp; the lower i-half (needs only early
    i0 exp + v) fills PE slack during the exp stream.
  - Output leaves as [p, itile, h] packed halves (4KB DMA lines), host
    unpacks.  Load doorbells: weights on GpSimd queue, X on Sync
    (each dma_start costs ~680ns of issue time on its queue).
"""

import sys

if "/opt/trn_rl_repo" not in sys.path:
    sys.path.insert(0, "/opt/trn_rl_repo")

import numpy as np

import concourse.bass as bass
import concourse.tile as tile
from concourse import bacc, mybir
from concourse.bass_utils import run_bass_kernel_spmd

P = 128          # partitions
S = 2048         # sequence length (per core)
D = 1024         # input dim
H = 128          # head dim (Dq = Dk)
ST = S // P      # 16 s-tiles
DC = D // P      # 8 d-chunks
NQ = 4           # s-quarters
QS = S // NQ     # 512
N_CORES = 8

F32 = mybir.dt.float32
BF16 = mybir.dt.bfloat16
AF = mybir.ActivationFunctionType

SOFTMAX_SCALE = 1.0 / float(np.sqrt(H))


def _build_kernel(tc, ins, out_ap):
    nc = tc.nc
    (qp, kp, vp, wq_ap, bq_ap, wk_ap, bk_ap, wv_ap, bv_ap) = ins

    with (
        tc.tile_pool(name="consts", bufs=1) as consts,
        tc.tile_pool(name="proj", bufs=1) as projp,
        tc.tile_pool(name="expp", bufs=1) as expp,
        tc.tile_pool(name="vext", bufs=1) as vexp,
        tc.tile_pool(name="accp", bufs=1) as accp,
        tc.tile_pool(name="outp", bufs=1) as outp,
        tc.tile_pool(name="xq", bufs=4) as xqp,
        tc.tile_pool(name="xk", bufs=4) as xkp,
        tc.tile_pool(name="xv", bufs=4) as xvp,
    ):
        # ---- tiny consts (no DMA) ----
        warm_a = consts.tile([P, P], BF16, tag="warm_a")
        nc.gpsimd.memset(warm_a, 0.5)
        warm_sink = nc.dram_tensor("warm_sink", [P, P], F32)

        # ---- load doorbells: weights/biases on GpSimd, X on Sync ----
        wq = consts.tile([P, DC, H], BF16, tag="wq")
        nc.gpsimd.dma_start(out=wq, in_=wq_ap)
        bq = consts.tile([P, 1], F32, tag="bq")
        nc.gpsimd.dma_start(out=bq, in_=bq_ap)
        wk = consts.tile([P, DC, H], BF16, tag="wk")
        nc.gpsimd.dma_start(out=wk, in_=wk_ap)
        bk = consts.tile([P, 1], F32, tag="bk")
        nc.gpsimd.dma_start(out=bk, in_=bk_ap)
        wv = consts.tile([P, DC, H], BF16, tag="wv")
        nc.gpsimd.dma_start(out=wv, in_=wv_ap)
        bvr = consts.tile([P, H], F32, tag="bvr")
        nc.gpsimd.dma_start(out=bvr, in_=bv_ap)

        xq_tiles = [
            xqp.tile([P, DC, QS], BF16, tag="xq", name=f"xq{nq}")
            for nq in range(NQ)
        ]
        xk_tiles = [
            xkp.tile([P, DC, QS], BF16, tag="xk", name=f"xk{t}")
            for t in range(NQ)
        ]
        xv_tiles = [
            xvp.tile([P, DC, QS], BF16, tag="xv", name=f"xv{nq}")
            for nq in range(NQ)
        ]
        # byte-priority: q half0 + k feed the exp stream, v is last
        nc.sync.dma_start(out=xq_tiles[0], in_=qp[0])
        nc.sync.dma_start(out=xq_tiles[1], in_=qp[1])
        for t in range(NQ):
            nc.sync.dma_start(out=xk_tiles[t], in_=kp[t])
        nc.sync.dma_start(out=xq_tiles[2], in_=qp[2])
        nc.sync.dma_start(out=xq_tiles[3], in_=qp[3])
        for t in range(NQ):
            nc.sync.dma_start(out=xv_tiles[t], in_=vp[t])

        # preload the ACT exp table set (~2.7us) during DMA dead time
        dummy = consts.tile([P, 1], F32, tag="dummy")
        nc.gpsimd.memset(dummy, 0.0)
        exp_sink = consts.tile([P, 1], BF16, tag="exp_sink")
        nc.scalar.activation(exp_sink, dummy, AF.Exp, bias=0.0, scale=1.0)

        # ---- persistent SBUF tiles ----
        qTq = [
            projp.tile([P, QS], BF16, tag=f"qT{i}", name=f"qT{i}")
            for i in range(NQ)
        ]
        kTq = [
            projp.tile([P, QS], BF16, tag=f"kT{i}", name=f"kT{i}")
            for i in range(NQ)
        ]
        ex = [
            [
                expp.tile([P, 4, 1024], BF16, tag=f"ex{h}{jq}", name=f"ex{h}{jq}")
                for jq in range(NQ)
            ]
            for h in range(2)
        ]
        vx = [
            vexp.tile([P, 4, H + 1], BF16, tag=f"vx{jq}", name=f"vx{jq}")
            for jq in range(NQ)
        ]
        for jq in range(NQ):
            nc.gpsimd.memset(vx[jq][:, :, H : H + 1], 1.0)
        acc = accp.tile([P, ST, H + 4], F32, tag="acc")
        rc_all = accp.tile([P, ST], F32, tag="rc_all")
        out_sb = [
            outp.tile([P, 8, H], BF16, tag=f"osb{hf}", name=f"osb{hf}")
            for hf in range(2)
        ]

        with (
            tc.tile_pool(name="psS", bufs=2, space="PSUM") as psS,   # 2x2 banks
            tc.tile_pool(name="psP", bufs=2, space="PSUM") as psP,   # 2x1 banks
            tc.tile_pool(name="psB", bufs=2, space="PSUM") as psB,   # 2x1 banks
        ):
            # ---- PE warm-up (HAM clock ramp needs sustained activity) ----
            ps_w = psP.tile([P, QS], F32, tag="pp", name="ps_w")
            for _ in range(90):
                nc.tensor.matmul(
                    ps_w[:, 0:P], warm_a, warm_a, start=True, stop=True
                )
            warm_sb = consts.tile([P, P], F32, tag="warm_sb")
            nc.vector.tensor_copy(warm_sb, ps_w[:, 0:P])
            nc.sync.dma_start(out=warm_sink[:, :], in_=warm_sb)

            def pe_keepalive(n):
                for _ in range(n):
                    nc.tensor.matmul(
                        ps_w[:, 0:P], warm_a, warm_a, start=True, stop=True
                    )

            def proj_quarter(xt, w, b, dst):
                ps = psP.tile([P, QS], F32, tag="pp")
                for dc in range(DC):
                    nc.tensor.matmul(
                        ps,
                        w[:, dc, :],
                        xt[:, dc, :],
                        start=(dc == 0),
                        stop=(dc == DC - 1),
                    )
                nc.vector.tensor_scalar_add(dst, ps, b)

            def scores_exp(jt, hf):
                kt_sl = kTq[jt // 4][:, (jt % 4) * P : (jt % 4 + 1) * P]
                pss = psS.tile([P, 1024], F32, tag="ps")
                for nb in range(2):
                    nc.tensor.matmul(
                        pss[:, nb * QS : (nb + 1) * QS],
                        kt_sl,
                        qTq[2 * hf + nb],
                        start=True,
                        stop=True,
                    )
                nc.scalar.activation(
                    ex[hf][jt // 4][:, jt % 4, :],
                    pss,
                    AF.Exp,
                    bias=0.0,
                    scale=SOFTMAX_SCALE,
                )

            # v quarter projection split into s-tile emission chunks so
            # it can fill PE slack between ACT-paced scores
            vps = {}

            def vproj_stile(jq, st):
                if jq not in vps:
                    vps[jq] = psP.tile([P, QS], F32, tag="pp", name=f"vps{jq}")
                ps = vps[jq]
                for dc in range(DC):
                    nc.tensor.matmul(
                        ps[:, st * P : (st + 1) * P],
                        xv_tiles[jq][:, dc, st * P : (st + 1) * P],
                        wv[:, dc, :],
                        start=(dc == 0),
                        stop=(dc == DC - 1),
                    )

            def vdrain(jq):
                nc.vector.tensor_copy(
                    vx[jq][:, :, 0:H],
                    vps[jq].rearrange("p (a b) -> p a b", b=P),
                )

            GROUPS = [(0, 3), (3, 3), (6, 2)]

            def av_group(ihalf, j0, nj, g0, glen, first):
                """AV partials: i-tiles [8ihalf+g0, +glen) x j-tiles
                [j0, j0+nj), 3 i-tiles per PSUM bank, one DVE drain."""
                i0 = 8 * ihalf
                po = psB.tile([P, 3, H + 4], F32, tag="po")
                for m in range(glen):
                    k = g0 + m
                    for dj in range(nj):
                        jt = j0 + dj
                        nc.tensor.matmul(
                            po[:, m, 0 : H + 1],
                            ex[ihalf][jt // 4][:, jt % 4, k * P : (k + 1) * P],
                            vx[jt // 4][:, jt % 4, :],
                            start=(dj == 0),
                            stop=(dj == nj - 1),
                        )
                dst = acc[:, i0 + g0 : i0 + g0 + glen, :]
                src = po[:, 0:glen, :]
                if first:
                    nc.vector.tensor_copy(dst, src)
                else:
                    nc.vector.tensor_add(dst, dst, src)

            def norm_store(ihalf):
                """Batched reciprocal; out = acc*rc + bv (bv folds in
                free since softmax rows sum to 1); one packed half DMA."""
                i0 = 8 * ihalf
                nc.vector.reciprocal(
                    rc_all[:, i0 : i0 + 8],
                    acc[:, i0 : i0 + 8, H : H + 1].squeeze(-1),
                )
                for g0, glen in GROUPS:
                    rc_bc = (
                        rc_all[:, i0 + g0 : i0 + g0 + glen]
                        .unsqueeze(-1)
                        .broadcast_to([P, glen, H])
                    )
                    dst = out_sb[ihalf][:, g0 : g0 + glen, :]
                    nc.vector.tensor_mul(
                        dst, acc[:, i0 + g0 : i0 + g0 + glen, 0:H], rc_bc
                    )
                    bv_bc = bvr[:, :].unsqueeze(1).broadcast_to(
                        [P, glen, H]
                    )
                    nc.vector.tensor_add(dst, dst, bv_bc)
                    nc.sync.dma_start(
                        out=out_ap[ihalf, :, g0 : g0 + glen, :], in_=dst
                    )

            # ---- emission order == intended engine execution order ----
            # PE is in-order: every insertion is placed at the point
            # where its data has just arrived, sized ~<=2us so the
            # ACT-paced scores stream never starves for long.
            proj_quarter(xq_tiles[0], wq, bq, qTq[0])
            proj_quarter(xq_tiles[1], wq, bq, qTq[1])
            pe_keepalive(20)
            for kq in range(NQ):
                proj_quarter(xk_tiles[kq], wk, bk, kTq[kq])
                for jt in range(4 * kq, 4 * kq + 4):
                    scores_exp(jt, 0)
                    if jt == 10:
                        proj_quarter(xq_tiles[2], wq, bq, qTq[2])
                    elif jt == 12:
                        proj_quarter(xq_tiles[3], wq, bq, qTq[3])
                    elif jt == 13:
                        vproj_stile(0, 0)
                        vproj_stile(0, 1)
                    elif jt == 14:
                        vproj_stile(0, 2)
                        vproj_stile(0, 3)
                        vdrain(0)
                    elif jt == 15:
                        vproj_stile(1, 0)
                        vproj_stile(1, 1)

            for jt in range(ST):
                scores_exp(jt, 1)
                if jt == 0:
                    vproj_stile(1, 2)
                    vproj_stile(1, 3)
                    vdrain(1)
                elif jt == 1:
                    vproj_stile(2, 0)
                    vproj_stile(2, 1)
                elif jt == 2:
                    vproj_stile(2, 2)
                    vproj_stile(2, 3)
                    vdrain(2)
                elif jt == 3:
                    av_group(0, 0, 8, 0, 3, True)
                elif jt == 4:
                    av_group(0, 0, 8, 3, 3, True)
                elif jt == 5:
                    av_group(0, 0, 8, 6, 2, True)
                elif jt == 6:
                    vproj_stile(3, 0)
                    vproj_stile(3, 1)
                elif jt == 7:
                    vproj_stile(3, 2)
                    vproj_stile(3, 3)
                    vdrain(3)
                elif jt == 8:
                    av_group(0, 8, 8, 0, 3, False)
                elif jt == 9:
                    av_group(0, 8, 8, 3, 3, False)
                elif jt == 10:
                    av_group(0, 8, 8, 6, 2, False)
                elif jt == 11:
                    av_group(1, 0, 4, 0, 3, True)
                elif jt == 12:
                    av_group(1, 0, 4, 3, 3, True)
                elif jt == 13:
                    av_group(1, 0, 4, 6, 2, True)
                    av_group(1, 4, 4, 0, 3, False)
                elif jt == 14:
                    av_group(1, 4, 4, 3, 3, False)
                elif jt == 15:
                    av_group(1, 4, 4, 6, 2, False)
            for g0, glen in GROUPS:
                av_group(1, 8, 4, g0, glen, False)
            norm_store(0)
            for g0, glen in GROUPS:
                av_group(1, 12, 4, g0, glen, False)
            norm_store(1)


def build_nc():
    nc = bacc.Bacc(
        "TRN2", target_bir_lowering=False, debug=False, num_devices=N_CORES
    )
    ins = [
        nc.dram_tensor("qp", [NQ, P, DC, QS], BF16, kind="ExternalInput").ap(),
        nc.dram_tensor("kp", [NQ, P, DC, QS], BF16, kind="ExternalInput").ap(),
        nc.dram_tensor("vp", [NQ, P, DC, QS], BF16, kind="ExternalInput").ap(),
        nc.dram_tensor("wq", [P, DC, H], BF16, kind="ExternalInput").ap(),
        nc.dram_tensor("bq", [P, 1], F32, kind="ExternalInput").ap(),
        nc.dram_tensor("wk", [P, DC, H], BF16, kind="ExternalInput").ap(),
        nc.dram_tensor("bk", [P, 1], F32, kind="ExternalInput").ap(),
        nc.dram_tensor("wv", [P, DC, H], BF16, kind="ExternalInput").ap(),
        nc.dram_tensor("bv", [P, H], F32, kind="ExternalInput").ap(),
    ]
    # packed [half, p, it_in_half, h]; host unpacks to [S, H]
    out_ap = nc.dram_tensor("out", [2, P, 8, H], BF16, kind="ExternalOutput").ap()
    with tile.TileContext(nc) as tc:
        _build_kernel(tc, ins, out_ap)
    nc.compile()
    return nc


_NC_CACHE = None


def _get_nc():
    global _NC_CACHE
    if _NC_CACHE is None:
        _NC_CACHE = build_nc()
    return _NC_CACHE


def _pack_xt(x_f32, bf):
    """[S, D] f32 -> X^T packed [NQ, P, DC, QS] bf16 (8KB DMA lines)."""
    xt = np.ascontiguousarray(x_f32.astype(bf).T)          # [D, S]
    return np.ascontiguousarray(
        xt.reshape(DC, P, NQ, QS).transpose(2, 1, 0, 3)
    )


def _pack_w(w_f32, bf):
    """[D, H] f32 -> [P, DC, H] bf16 (2KB DMA lines)."""
    return np.ascontiguousarray(
        w_f32.astype(bf).reshape(DC, P, H).transpose(1, 0, 2)
    )


def _run(inputs, trace=False, **kw):
    import ml_dtypes

    nc = _get_nc()
    bf = np.dtype(ml_dtypes.bfloat16)
    q = np.asarray(inputs["query"], dtype=np.float32)
    k = np.asarray(inputs["key"], dtype=np.float32)
    v = np.asarray(inputs["value"], dtype=np.float32)
    shared = {
        "wq": _pack_w(np.asarray(inputs["Wq"], dtype=np.float32), bf),
        "wk": _pack_w(np.asarray(inputs["Wk"], dtype=np.float32), bf),
        "wv": _pack_w(np.asarray(inputs["Wv"], dtype=np.float32), bf),
        "bq": np.ascontiguousarray(
            np.asarray(inputs["bq"], dtype=np.float32).reshape(P, 1)
        ),
        "bk": np.ascontiguousarray(
            np.asarray(inputs["bk"], dtype=np.float32).reshape(P, 1)
        ),
        "bv": np.ascontiguousarray(
            np.broadcast_to(
                np.asarray(inputs["bv"], dtype=np.float32).reshape(1, H), (P, H)
            )
        ),
    }
    in_maps = [
        {
            "qp": _pack_xt(q[c], bf),
            "kp": _pack_xt(k[c], bf),
            "vp": _pack_xt(v[c], bf),
            **shared,
        }
        for c in range(N_CORES)
    ]
    res = run_bass_kernel_spmd(nc, in_maps, list(range(N_CORES)), trace=trace, **kw)
    # unpack [2, P, 8, H] -> [S, H]: s = 1024*half + 128*it + p
    out = np.stack(
        [
            res.results[c]["out"].transpose(0, 2, 1, 3).reshape(S, H)
            for c in range(N_CORES)
        ],
        axis=0,
    )
    return out.astype(np.float32), res


def kernel(**inputs) -> np.ndarray:
    out, _ = _run(inputs, trace=False)
    return out


if __name__ == "__main__":
    # smoke-build only
    build_nc()
    print("build ok")
